# revision 12
# baseline (speedup 1.0000x reference)
"""Trainium2 Bass kernel for nn_DAGLayer (gnn_message_passing).

Problem: out buffer holds L leaf columns followed by M computed nodes.
Node i gathers P=8 parent columns (each [N, C]) from the buffer, applies a
per-node dense map y = einsum('ncp,ocp->no', g, W[i]) + b[i], and appends y.

Strategy (8 NeuronCores, SPMD single program):
  - Host computes DAG levels from `parents`; nodes of one level are
    independent, so each level is executed as one parallel "round".
  - Within a round the nodes are dealt round-robin to the 8 cores
    (node-parallel; per-node weights live only on the owning core, so the
    1 GB weight tensor is sharded 8x - the kernel is weight-DMA bound).
  - Every core keeps a replicated history buffer `hbuf` in DRAM holding all
    node outputs as [slot, n, c] rows; after each round an AllGather
    concatenates the 8 cores' new outputs into everyone's hbuf.
  - Parent gathers use indirect DMA with per-core offset tables (the single
    SPMD program reads different rows on each core purely via input data).
  - Per node: gather 2x[128,256] (rows = (tap,n), cols = c), PE-transpose to
    get contraction (tap,c) onto partitions, then 16 accumulating matmuls
    [128k x 32n] x [128k x 256o] into PSUM [32, 256] plus one K=1 matmul to
    add the bias. float32r dtype gives 1 cycle/row on the 256-wide moving
    operand (plain fp32 would be 4).

The kernel is self-contained: shapes are taken from the inputs, the schedule
is derived from `parents` at run time on the host.
"""

import os

import numpy as np

os.environ.setdefault("NEURON_COMPILE_CACHE_URL", "/root/neuron_cache")

NCORES = 8

_BUILD_CACHE = {}


def _compute_levels(parents, L, M):
    lvl = np.zeros(L + M, np.int64)
    pare = np.asarray(parents, np.int64)
    for i in range(M):
        lvl[L + i] = 1 + lvl[pare[i]].max()
    nlev = int(lvl[L:].max()) if M else 0
    level_nodes = [np.nonzero(lvl[L:] == d)[0] for d in range(1, nlev + 1)]
    return level_nodes


def _build_bass(L, s_list, S, total_slots):
    import concourse.bass as bass
    import concourse.bacc as bacc
    import concourse.mybir as mybir
    import concourse.tile as tile
    from concourse.masks import make_identity

    f32r = mybir.dt.float32r
    f32 = mybir.dt.float32
    i32 = mybir.dt.int32

    nc = bacc.Bacc(num_devices=NCORES)

    wbuf = nc.dram_tensor("wbuf", [S, 128, 16, 256], f32r, kind="ExternalInput")
    xt = nc.dram_tensor("xt", [L * 32, 256], f32r, kind="ExternalInput")
    # bbuf packs a [1, 32] ones-vector (for the K=1 bias matmul lhsT) followed
    # by the S per-node bias rows.
    bbuf = nc.dram_tensor("bbuf", [1, 32 + S * 256], f32r, kind="ExternalInput")
    gidx = nc.dram_tensor("gidx", [128, 2 * S], i32, kind="ExternalInput")
    yout = nc.dram_tensor("yout", [S * 32, 256], f32r, kind="ExternalOutput")
    hbuf = nc.dram_tensor("hbuf", [total_slots * 32, 256], f32r, addr_space="Shared")
    agin = nc.dram_tensor("agin", [S * 32, 256], f32r)
    rg = [list(range(NCORES))]

    with tile.TileContext(nc) as tc:
        with (
            tc.tile_pool(name="const", bufs=1) as constp,
            tc.tile_pool(name="w", bufs=4) as wp,
            tc.tile_pool(name="g", bufs=6) as gp,
            tc.tile_pool(name="gt", bufs=3) as gtp,
            tc.tile_pool(name="y", bufs=3) as yp,
            tc.tile_pool(name="ptr", bufs=4, space="PSUM") as ptrp,
            tc.tile_pool(name="py", bufs=2, space="PSUM") as pyp,
        ):
            ident_f = constp.tile([128, 128], f32)
            make_identity(nc, ident_f[:])
            ident = constp.tile([128, 128], f32r)
            nc.scalar.copy(ident[:], ident_f[:])
            b_sb = constp.tile([1, 32 + S * 256], f32r)
            nc.sync.dma_start(b_sb[:], bbuf[:])
            ones = b_sb[:, 0:32]
            gidx_sb = constp.tile([128, 2 * S], i32)
            nc.sync.dma_start(gidx_sb[:], gidx[:])
            # leaves into the shared history buffer
            nc.gpsimd.dma_start(hbuf[0 : L * 32, :], xt[:])

            off = 0
            for s_r in s_list:
                for m in range(s_r):
                    s = off + m
                    w_t = wp.tile([128, 16, 256], f32r, tag="w")
                    nc.sync.dma_start(w_t[:], wbuf[s])

                    g0 = gp.tile([128, 256], f32r, tag="g")
                    g1 = gp.tile([128, 256], f32r, tag="g")
                    for t, g_t in enumerate((g0, g1)):
                        nc.gpsimd.indirect_dma_start(
                            out=g_t[:],
                            out_offset=None,
                            in_=hbuf[:, :],
                            in_offset=bass.IndirectOffsetOnAxis(
                                ap=gidx_sb[:, 2 * s + t : 2 * s + t + 1], axis=0
                            ),
                        )
                    gT = gtp.tile([128, 4, 128], f32r, tag="gt")
                    for t, g_t in enumerate((g0, g1)):
                        for h in range(2):
                            ptr = ptrp.tile([128, 128], f32r, tag="ptr")
                            nc.tensor.transpose(
                                ptr[:], g_t[:, h * 128 : (h + 1) * 128], ident[:]
                            )
                            nc.vector.tensor_copy(gT[:, 2 * t + h, :], ptr[:])
                    py = pyp.tile([32, 256], f32, tag="py")
                    for kk in range(16):
                        tap, h = divmod(kk, 2)
                        t, lt = divmod(tap, 4)
                        nc.tensor.matmul(
                            py[:],
                            gT[:, 2 * t + h, lt * 32 : (lt + 1) * 32],
                            w_t[:, kk, :],
                            start=(kk == 0),
                            stop=False,
                        )
                    nc.tensor.matmul(
                        py[:],
                        ones,
                        b_sb[:, 32 + s * 256 : 32 + (s + 1) * 256],
                        start=False,
                        stop=True,
                    )
                    y_sb = yp.tile([32, 256], f32r, tag="y")
                    nc.scalar.copy(y_sb[:], py[:])
                    nc.sync.dma_start(agin[s * 32 : (s + 1) * 32, :], y_sb[:])
                    nc.sync.dma_start(yout[s * 32 : (s + 1) * 32, :], y_sb[:])
                gbase = L + 8 * off
                nc.gpsimd.collective_compute(
                    "AllGather",
                    mybir.AluOpType.bypass,
                    replica_groups=rg,
                    ins=[agin[off * 32 : (off + s_r) * 32, :]],
                    outs=[hbuf[gbase * 32 : (gbase + 8 * s_r) * 32, :]],
                )
                off += s_r
    nc.compile()
    return nc


def kernel(x, W, b, parents):
    from concourse.bass_utils import run_bass_kernel_spmd

    x = np.ascontiguousarray(np.asarray(x), dtype=np.float32)
    W = np.ascontiguousarray(np.asarray(W), dtype=np.float32)
    b = np.ascontiguousarray(np.asarray(b), dtype=np.float32)
    parents = np.asarray(parents).astype(np.int64)

    N, C, L = x.shape
    M, O, C2, P = W.shape
    assert (N, C, O, C2, P) == (32, 256, 256, 256, 8), "kernel hardcodes these dims"

    level_nodes = _compute_levels(parents, L, M)
    s_list = [(len(nodes) + NCORES - 1) // NCORES for nodes in level_nodes]
    S = sum(s_list)
    total_slots = L + 8 * S

    # slot assignment: round r occupies global slots [L+8*off_r, L+8*(off_r+s_r))
    # in AllGather rank-major order; core q's m-th slot of round r holds the
    # (q + 8*m)-th node of the level.
    slot_of = np.full(L + M, -1, np.int64)
    slot_of[:L] = np.arange(L)
    node_of_coreslot = np.full((NCORES, S), -1, np.int64)
    off = 0
    for r, nodes in enumerate(level_nodes):
        s_r = s_list[r]
        for j, node in enumerate(nodes):
            q, m = j % NCORES, j // NCORES
            slot_of[L + node] = L + 8 * off + q * s_r + m
            node_of_coreslot[q, off + m] = node
        off += s_r
    assert (slot_of >= 0).all()

    # weight relayout: [M, o, c, p] -> [M, 128(part), 16(ktile), 256(o)]
    # with k = tap*256 + c, partition = k % 128, ktile = k // 128.
    W3 = np.ascontiguousarray(
        W.transpose(0, 3, 2, 1).reshape(M, 16, 128, 256).transpose(0, 2, 1, 3)
    )
    xt_host = np.ascontiguousarray(x.transpose(2, 0, 1).reshape(L * 32, 256))

    narange = np.arange(32, dtype=np.int64)
    in_maps = []
    for q in range(NCORES):
        nodes_q = node_of_coreslot[q]
        valid = nodes_q >= 0
        Wq = np.zeros((S, 128, 16, 256), np.float32)
        Wq[valid] = W3[nodes_q[valid]]
        bq = np.zeros((S, 256), np.float32)
        bq[valid] = b[nodes_q[valid]]
        bq_packed = np.concatenate(
            [np.ones(32, np.float32), bq.reshape(-1)]
        ).reshape(1, 32 + S * 256)
        gq = np.zeros((128, 2 * S), np.int32)
        for s in range(S):
            node = nodes_q[s]
            par = parents[node] if node >= 0 else np.zeros(P, np.int64)
            pslots = slot_of[par]
            for t in range(2):
                for lt in range(4):
                    gq[lt * 32 : (lt + 1) * 32, 2 * s + t] = (
                        pslots[4 * t + lt] * 32 + narange
                    )
        in_maps.append(
            {
                "wbuf": Wq,
                "xt": xt_host,
                "bbuf": np.ascontiguousarray(bq_packed),
                "gidx": gq,
            }
        )

    key = (L, tuple(s_list))
    if key not in _BUILD_CACHE:
        import time as _time

        _t0 = _time.time()
        _BUILD_CACHE[key] = _build_bass(L, s_list, S, total_slots)
        print(f"[kernel] bass build took {_time.time() - _t0:.1f}s", flush=True)
    nc = _BUILD_CACHE[key]

    global LAST_RUN
    LAST_RUN = (nc, in_maps)

    results = run_bass_kernel_spmd(nc, in_maps, core_ids=list(range(NCORES))).results

    out = np.zeros((N, C, L + M), np.float32)
    out[:, :, :L] = x
    for q in range(NCORES):
        yq = np.asarray(results[q]["yout"]).reshape(S, 32, 256)
        for s in range(S):
            node = node_of_coreslot[q, s]
            if node >= 0:
                out[:, :, L + node] = yq[s]
    return out
